# revision 12
# baseline (speedup 1.0000x reference)
"""Pairwise squared-Euclidean distance matrix kernel for Trainium2.

Computes D[b, i, j] = ||A[b,i] - B[b,j]||^2 for A, B of shape [16, 4096, 256]
fp32, returning [16, 4096, 4096] fp32.

Sharding: data-parallel over the batch dim -- 2 batches per NeuronCore over
8 cores (SPMD: same program, different batch slices).

Design (v4):
  * Output is written to DRAM as bf16 and upcast to fp32 on the host.
    Halves the dominant HBM write stream (134 MB -> 67 MB per core).
  * The cross term -2*A.B is computed in fp8e4 (e4m3) with
    perf_mode=DoubleRow: one matmul per 512-wide j-tile contracts the
    full K=256. The B^T chunks are stored k-pair INTERLEAVED
    ([P, NT, 2] memory, presented as [P, 2, NT] strided AP) so the
    moving-operand stream is contiguous per partition. The -2 is folded
    into the fp8 cast of A^T.
  * rB comes from exact fp32 squares of natural-layout B (ScalarE
    square + accum), downcast to bf16, then a DRAM round-trip
    (scatter + partition-broadcast load) lands it broadcast in SBUF.
  * The epilogue (PSUM -> +rA +rB -> bf16 SBUF) works on [128, 1024]
    j-tile PAIRS split across engines; per-pair kinds:
      - dvep: DVE scalar_tensor_tensor  out = (psum + rA) + rB
      - gpp:  ScalarE activation(Identity, bias=rA) -> bf16 tmp;
              GpSimd tensor_add adds broadcast rB (GpSimd cannot
              read PSUM, hence the two stages).
      - actp: rB is folded into PSUM by a 1024-wide ones(1/128)-matmul
              over the partition-broadcast rB; ScalarE bias-copy then
              writes the final bf16 slice directly.
    Mix per 4-row cycle: 8 dvep / 5 gpp / 3 actp pair-slots, rotated,
    balancing measured rates (DVE 1.28us, ACT 1.05us, GP 2.12us,
    PE +0.43us per pair).
  * B^T fp8 chunk casts (PSUM -> SBUF) run on DVE tensor_copy to keep
    ScalarE under budget.

Error budget: fp8e4 cross term ~1.1 rms, bf16 output quant ~1.0 rms,
bf16 rB ~0.3 rms on |D| ~ 512 scale: rel l2 ~ 3e-3.
"""

from contextlib import ExitStack

import numpy as np

import concourse.mybir as mybir
import concourse.tile as tile
from concourse import bacc
from concourse.bass import ts
from concourse.masks import make_identity

F32 = mybir.dt.float32
BF16 = mybir.dt.bfloat16
FP8 = mybir.dt.float8e4
AF = mybir.ActivationFunctionType
ALU = mybir.AluOpType

N_CORES = 8
FULL_BATCH = 16
N = 4096
D = 256
P = 128
NT = 512  # output j-tile width (one PSUM bank of fp32)
LOADG = 4  # natural-layout tiles coalesced per input DMA (= NT/P)
INTERLEAVE_BT = True  # store B^T chunks k-pair-interleaved for DoubleRow


def make_row_plans(n_jtiles):
    """Cycle of per-row epilogue plans; each plan is a list of
    (kind, j0) pair items covering j-tiles j0, j0+1."""
    if n_jtiles == 1:
        return [[("dve1", 0)]]
    if n_jtiles == 2:
        return [[("dvep", 0)], [("gpp", 0)], [("actp", 0)]]
    assert n_jtiles % 2 == 0
    kinds_cycle = [
        ["gpp", "dvep", "actp", "dvep"],
        ["dvep", "gpp", "dvep", "gpp"],
        ["actp", "dvep", "gpp", "dvep"],
        ["dvep", "actp", "dvep", "gpp"],
    ]
    plans = []
    for kinds in kinds_cycle:
        plan = []
        for i, j0 in enumerate(range(0, n_jtiles, 2)):
            plan.append((kinds[i % 4], j0))
        plans.append(plan)
    return plans


def build_nc(b_per_core=FULL_BATCH // N_CORES, n=N, d=D):
    n_itiles = n // P
    n_jtiles = n // NT
    n_ktiles = d // P
    t_per_j = NT // P  # B tiles per bt chunk
    assert n_ktiles == 2, "DoubleRow packing assumes K = 2*128"
    assert LOADG == t_per_j

    plans = make_row_plans(n_jtiles)

    nc = bacc.Bacc()
    a_ext = nc.declare_dram_parameter("A", [b_per_core, n, d], F32, isOutput=False)
    b_ext = nc.declare_dram_parameter("B", [b_per_core, n, d], F32, isOutput=False)
    d_ext = nc.declare_dram_parameter("D", [b_per_core, n, n], BF16, isOutput=True)

    with tile.TileContext(nc) as tc, ExitStack() as ctx:
        const_pool = ctx.enter_context(tc.tile_pool(name="const", bufs=1))
        nat_pool = ctx.enter_context(tc.tile_pool(name="nat", bufs=3))
        sqa_pool = ctx.enter_context(tc.tile_pool(name="sqa", bufs=2))
        sqb_pool = ctx.enter_context(tc.tile_pool(name="sqb", bufs=2))
        bt_pool = ctx.enter_context(tc.tile_pool(name="bt", bufs=2 * n_jtiles))
        rb_pool = ctx.enter_context(
            tc.tile_pool(name="rb", bufs=2 * max(n_jtiles // 2, 1))
        )
        rbg_pool = ctx.enter_context(tc.tile_pool(name="rbg", bufs=4))
        at_pool = ctx.enter_context(tc.tile_pool(name="at", bufs=6))
        ra_pool = ctx.enter_context(tc.tile_pool(name="ra", bufs=8))
        tmp_pool = ctx.enter_context(tc.tile_pool(name="tmp", bufs=4))
        out_pool = ctx.enter_context(tc.tile_pool(name="out", bufs=8))
        dram_pool = ctx.enter_context(tc.tile_pool(name="dram", bufs=2, space="DRAM"))
        # PSUM: 8 banks of [P, 512] fp32. 3x2 (pairs, shared by matmul
        # accumulation and the rB build) + 2x1 (transposes) = 8
        psum_pair = ctx.enter_context(tc.tile_pool(name="psum_pair", bufs=3, space="PSUM"))
        psum_tr = ctx.enter_context(tc.tile_pool(name="psum_tr", bufs=2, space="PSUM"))

        ident = const_pool.tile([P, P], F32)
        make_identity(nc, ident)
        # 1/128-valued bf16 [P, P]: partition-reduction matmul over the
        # partition-broadcast rB reproduces rB (for the actp fold)
        ones_f = const_pool.tile([P, P], F32)
        nc.scalar.activation(ones_f[:], ident[:], AF.Identity, bias=1.0, scale=0.0)
        ones_b = const_pool.tile([P, P], BF16)
        nc.scalar.mul(ones_b[:], ones_f[:], 1.0 / P)

        bt_chunks = {}  # (b, jt) -> fp8 B^T chunk tile
        rb_pairs = {}  # (b, jp) -> [P, 2, NT] bf16 broadcast rB for jt 2jp, 2jp+1

        GW = LOADG * P  # j-width covered by one B group (== NT)
        n_bgroups = n_itiles // LOADG
        n_agroups = n_itiles // LOADG

        def chunk_kview(chunk):
            """[P, 2, NT] k-major view of a B^T chunk."""
            if INTERLEAVE_BT:
                return chunk[:, :, :].rearrange("p n two -> p two n")
            return chunk[:, :, :]

        def emit_b_group(b, g):
            """Load one 512-wide B panel slice; PE-transpose it into an fp8
            chunk (DVE casts); exact fp32 squares + DRAM round-trip for rB."""
            bn = nat_pool.tile([P, LOADG, d], F32, tag="bn")
            nc.gpsimd.dma_start(
                bn[:],
                b_ext[b, ts(g, GW), :].rearrange("(t p) d -> p t d", p=P),
            )
            if INTERLEAVE_BT:
                chunk = bt_pool.tile([P, NT, n_ktiles], FP8, tag="bt", name="bt_chunk")
            else:
                chunk = bt_pool.tile([P, n_ktiles, NT], FP8, tag="bt", name="bt_chunk")
            for tt in range(t_per_j):
                ps = psum_tr.tile([P, 2, P], F32, tag="ps_tr")
                for k in range(n_ktiles):
                    nc.tensor.transpose(ps[:, k, :], bn[:, tt, ts(k, P)], ident)
                if INTERLEAVE_BT:
                    dst = chunk[:, ts(tt, P), :].rearrange("p n two -> p two n")
                else:
                    dst = chunk[:, :, ts(tt, P)]
                nc.vector.tensor_copy(dst, ps[:])
            # rB: exact fp32 squares of natural B, accumulate per tile
            sqb = sqb_pool.tile([P, LOADG, d], BF16, tag="sqb")
            r_bg = rbg_pool.tile([P, LOADG], F32, tag="rbg", name="r_bg")
            for tt in range(t_per_j):
                nc.scalar.activation(
                    sqb[:, tt], bn[:, tt], AF.Square, accum_out=r_bg[:, tt : tt + 1]
                )
            r_bg16 = rbg_pool.tile([P, LOADG], BF16, tag="rbg16", name="r_bg16")
            nc.scalar.copy(r_bg16[:], r_bg[:])
            jp, half = divmod(g, 2)
            if half == 0:
                rb_pairs[(b, jp)] = rb_pool.tile(
                    [P, 2, NT], BF16, tag="rb", name="rb_pair"
                )
            rb_dram = dram_pool.tile([GW], BF16, tag="rb_dram", name="rb_dram")
            nc.sync.dma_start(rb_dram[:].rearrange("(t p) -> p t", p=P), r_bg16[:])
            nc.sync.dma_start(
                rb_pairs[(b, jp)][:, half, :], rb_dram[:].partition_broadcast(P)
            )
            bt_chunks[(b, g)] = chunk

        def load_a_group(b, g):
            t = nat_pool.tile([P, LOADG, d], F32, tag="an", name="an_group")
            nc.gpsimd.dma_start(
                t[:],
                a_ext[b, ts(g, LOADG * P), :].rearrange("(t p) d -> p t d", p=P),
            )
            return t

        def emit_a_row_pre(an):
            """rA square + A^T transpose and -2x fp8 cast for one row."""
            r_a = ra_pool.tile([P, 1], F32, tag="rA", name="r_a")
            sqa = sqa_pool.tile([P, d], BF16, tag="sqa")
            nc.scalar.activation(sqa[:], an, AF.Square, accum_out=r_a[:])
            at_tile = at_pool.tile([P, n_ktiles, P], FP8, tag="at", name="at_tile")
            ps = psum_tr.tile([P, 2, P], F32, tag="ps_tr")
            for k in range(n_ktiles):
                nc.tensor.transpose(ps[:, k, :], an[:, ts(k, P)], ident)
            # fold the -2 of "-2*a.b" into the fp8 cast of A^T (one op)
            nc.scalar.mul(at_tile[:, :, :], ps[:], -2.0)
            return r_a, at_tile

        def mm_cross(out_ps, b, jt, at_tile, start=True, stop=True, skip=False):
            """One DoubleRow fp8 matmul: full K=256 cross term for a j-tile."""
            nc.tensor.matmul(
                out_ps,
                lhsT=at_tile[:, :, :],
                rhs=chunk_kview(bt_chunks[(b, jt)]),
                start=start,
                stop=stop,
                perf_mode=mybir.MatmulPerfMode.DoubleRow,
                skip_group_check=skip,
            )

        def emit_item(b, item, r_a, at_tile, out_row):
            kind, j0 = item
            mm_ps = psum_pair.tile([P, 2 * NT], F32, tag="mm_pair", name="mm_pair")
            rbp = rb_pairs[(b, j0 // 2)]
            if kind == "dve1":  # tiny configs: single j-tile via DVE
                mm_cross(mm_ps[:, :NT], b, j0, at_tile)
                nc.vector.scalar_tensor_tensor(
                    out=out_row[:, ts(j0, NT)],
                    in0=mm_ps[:, :NT],
                    scalar=r_a[:],
                    in1=rbp[:, j0 % 2, :],
                    op0=ALU.add,
                    op1=ALU.add,
                )
                return
            if kind == "actp":
                # fold rB into PSUM: ones(1/128).T @ broadcast-rB, 1024 wide
                for jj in range(2):
                    mm_cross(
                        mm_ps[:, ts(jj, NT)], b, j0 + jj, at_tile,
                        start=True, stop=False, skip=True,
                    )
                    nc.tensor.matmul(
                        mm_ps[:, ts(jj, NT)],
                        lhsT=ones_b[:],
                        rhs=rbp[:, jj, :],
                        start=False,
                        stop=True,
                        skip_group_check=True,
                    )
                nc.scalar.activation(
                    out_row[:, j0 * NT : (j0 + 2) * NT],
                    mm_ps[:],
                    AF.Identity,
                    bias=r_a[:],
                    scale=1.0,
                )
                return
            for jj in range(2):
                mm_cross(mm_ps[:, ts(jj, NT)], b, j0 + jj, at_tile)
            if kind == "dvep":
                nc.vector.scalar_tensor_tensor(
                    out=out_row[:, j0 * NT : (j0 + 2) * NT],
                    in0=mm_ps[:],
                    scalar=r_a[:],
                    in1=rbp[:, :, :],
                    op0=ALU.add,
                    op1=ALU.add,
                )
            else:  # "gpp": ScalarE evacuates psum with +rA; GpSimd adds rB
                tmp = tmp_pool.tile([P, 2 * NT], BF16, tag="tmp", name="act_tmp")
                nc.scalar.activation(
                    tmp[:], mm_ps[:], AF.Identity, bias=r_a[:], scale=1.0
                )
                nc.gpsimd.tensor_add(
                    out_row[:, j0 * NT : (j0 + 2) * NT],
                    tmp[:],
                    rbp[:, :, :].rearrange("p two n -> p (two n)"),
                )

        an_groups = {0: load_a_group(0, 0)}

        # --- batch-0 startup: first LOADG rows emitted j-outer, interleaved
        # with the B preprocess, so output DMAs start as soon as chunks land.
        pre_rows = min(LOADG, n_itiles)
        pre = [emit_a_row_pre(an_groups[0][:, r]) for r in range(pre_rows)]
        if n_agroups > 1 or b_per_core > 1:
            gnext = 1 % n_agroups
            an_groups[gnext] = load_a_group(0 if n_agroups > 1 else 1, gnext)
        pre_outs = [
            out_pool.tile([P, n], BF16, tag="out_row", name="out_row")
            for _ in range(pre_rows)
        ]
        # warmup rows r use plans[r % len(plans)]; emit each item as soon as
        # its last B chunk (group j0+1, or j0 for single) is processed
        for g in range(n_bgroups):
            emit_b_group(0, g)
            for r in range(pre_rows):
                for item in plans[r % len(plans)]:
                    last_g = item[1] + (0 if item[0] == "dve1" else 1)
                    if last_g == g:
                        emit_item(0, item, pre[r][0], pre[r][1], pre_outs[r])
        for r in range(pre_rows):
            nc.sync.dma_start(d_ext[0, ts(r, P), :], pre_outs[r][:])

        # --- main loop
        b_emitted = {0: n_bgroups}  # batch -> number of B groups emitted
        for b in range(b_per_core):
            for g in range(b_emitted.get(b, 0), n_bgroups):
                emit_b_group(b, g)  # catch-up (only for tiny configs)
                b_emitted[b] = g + 1
            for it in range(pre_rows if b == 0 else 0, n_itiles):
                # spread next batch's B preprocess across early iterations
                if b + 1 < b_per_core:
                    it0 = it - (pre_rows if b == 0 else 0)
                    if it0 < n_bgroups:
                        emit_b_group(b + 1, it0)
                        b_emitted[b + 1] = it0 + 1

                g, ti = divmod(it, LOADG)
                if ti == 0:
                    # prefetch the next A group one group ahead
                    if g + 1 < n_agroups:
                        an_groups[g + 1] = load_a_group(b, g + 1)
                    elif b + 1 < b_per_core:
                        an_groups[0] = load_a_group(b + 1, 0)
                an = an_groups[g][:, ti]
                r_a, at_tile = emit_a_row_pre(an)
                out_row = out_pool.tile([P, n], BF16, tag="out_row")
                for item in plans[it % len(plans)]:
                    emit_item(b, item, r_a, at_tile, out_row)
                nc.sync.dma_start(d_ext[b, ts(it, P), :], out_row[:])

    nc.compile()
    return nc


_NC_CACHE = {}


def _get_nc(b_per_core, n, d):
    key = (b_per_core, n, d)
    if key not in _NC_CACHE:
        _NC_CACHE[key] = build_nc(b_per_core, n, d)
    return _NC_CACHE[key]


def run(A, B, trace=False, trace_kwargs=None):
    """Run on hardware across 8 cores; returns (D_full, BassKernelResults)."""
    from concourse.bass_utils import run_bass_kernel_spmd

    A = np.ascontiguousarray(np.asarray(A, dtype=np.float32))
    B = np.ascontiguousarray(np.asarray(B, dtype=np.float32))
    full_b = A.shape[0]
    assert full_b % N_CORES == 0
    bpc = full_b // N_CORES
    nc = _get_nc(bpc, A.shape[1], A.shape[2])

    in_maps = [
        {
            "A": A[c * bpc : (c + 1) * bpc],
            "B": B[c * bpc : (c + 1) * bpc],
        }
        for c in range(N_CORES)
    ]
    res = run_bass_kernel_spmd(
        nc,
        in_maps,
        list(range(N_CORES)),
        trace=trace,
        **(trace_kwargs or {}),
    )
    out = np.concatenate(
        [np.asarray(r["D"]).astype(np.float32) for r in res.results], axis=0
    )
    return out, res


def kernel(A, B):
    out, _ = run(A, B, trace=False)
    return out


# revision 13
# speedup vs baseline: 1.7408x; 1.7408x over previous
"""Pairwise squared-Euclidean distance matrix kernel for Trainium2.

Computes D[b, i, j] = ||A[b,i] - B[b,j]||^2 for A, B of shape [16, 4096, 256]
fp32, returning [16, 4096, 4096] fp32.

Sharding: data-parallel over the batch dim -- 2 batches per NeuronCore over
8 cores (SPMD: same program, different batch slices).

Design (v5):
  * Output is written to DRAM as bf16 and upcast to fp32 on the host.
    Halves the dominant HBM write stream (134 MB -> 67 MB per core).
  * The cross term -2*A.B is computed in fp8e4 (e4m3) with
    perf_mode=DoubleRow: one matmul per 512-wide j-tile contracts the
    full K=256 ([128, 2, :] operand layout). The -2 is folded into the
    fp8 cast of A^T. (Measured ~380ns/MM warm -- the fp8 pair-rate
    fast path does not engage on this shape; still beats 2x bf16.)
  * rB is computed from bf16 squares of the (exact, fp32) PE-transposed B
    panel via an all-ones matmul (reduces over partitions), which lands rB
    already BROADCAST across partitions in PSUM -- no DRAM round-trip.
    (A DMA round-trip variant measured far worse: its scatter descriptors
    flood the SDMA queues and starve the output stream.)
  * The epilogue (PSUM -> +rA +rB -> bf16 SBUF) works on [128, 1024]
    j-tile PAIRS split across engines; per-pair kinds:
      - dvep: DVE scalar_tensor_tensor  out = (psum + rA) + rB
      - gpp:  ScalarE activation(Identity, bias=rA) -> bf16 tmp;
              GpSimd tensor_add adds broadcast rB (GpSimd cannot
              read PSUM, hence the two stages).
      - actp: rB is folded into PSUM by ones(1/128)-matmuls over the
              partition-broadcast rB; ScalarE bias-copy then writes the
              final bf16 slice directly (no GpSimd stage).
    Mix per 4-row cycle: 9 dvep / 5 gpp / 2 actp pair-slots, rotated,
    balancing measured rates (DVE 1.28us, ACT 1.05us, GP 2.12us,
    PE +0.43us per pair).
  * B^T fp8 chunk casts (PSUM -> SBUF) run on DVE tensor_copy to keep
    ScalarE under budget; ScalarE keeps the bf16 squares for rB.

Error budget: fp8e4 cross term ~1.1 rms, bf16 output quant ~1.0 rms,
bf16 rB ~0.3 rms on |D| ~ 512 scale: rel l2 ~ 3e-3.
"""

from contextlib import ExitStack

import numpy as np

import concourse.mybir as mybir
import concourse.tile as tile
from concourse import bacc
from concourse.bass import ts
from concourse.masks import make_identity

F32 = mybir.dt.float32
BF16 = mybir.dt.bfloat16
FP8 = mybir.dt.float8e4
AF = mybir.ActivationFunctionType
ALU = mybir.AluOpType

N_CORES = 8
FULL_BATCH = 16
N = 4096
D = 256
P = 128
NT = 512  # output j-tile width (one PSUM bank of fp32)
LOADG = 4  # natural-layout tiles coalesced per input DMA (= NT/P)


def make_row_plans(n_jtiles):
    """Cycle of per-row epilogue plans; each plan is a list of
    (kind, j0) pair items covering j-tiles j0, j0+1."""
    if n_jtiles == 1:
        return [[("dve1", 0)]]
    if n_jtiles == 2:
        return [[("dvep", 0)], [("gpp", 0)], [("actp", 0)]]
    assert n_jtiles % 2 == 0
    kinds_cycle = [
        ["dvep", "gpp", "actp", "dvep"],
        ["gpp", "dvep", "dvep", "gpp"],
        ["actp", "dvep", "gpp", "dvep"],
        ["dvep", "gpp", "dvep", "dvep"],
    ]
    plans = []
    for kinds in kinds_cycle:
        plan = []
        for i, j0 in enumerate(range(0, n_jtiles, 2)):
            plan.append((kinds[i % 4], j0))
        plans.append(plan)
    return plans


def build_nc(b_per_core=FULL_BATCH // N_CORES, n=N, d=D):
    n_itiles = n // P
    n_jtiles = n // NT
    n_ktiles = d // P
    t_per_j = NT // P  # B tiles per bt chunk
    assert n_ktiles == 2, "DoubleRow packing assumes K = 2*128"
    assert LOADG == t_per_j

    plans = make_row_plans(n_jtiles)

    nc = bacc.Bacc()
    a_ext = nc.declare_dram_parameter("A", [b_per_core, n, d], F32, isOutput=False)
    b_ext = nc.declare_dram_parameter("B", [b_per_core, n, d], F32, isOutput=False)
    d_ext = nc.declare_dram_parameter("D", [b_per_core, n, n], BF16, isOutput=True)

    with tile.TileContext(nc) as tc, ExitStack() as ctx:
        const_pool = ctx.enter_context(tc.tile_pool(name="const", bufs=1))
        nat_pool = ctx.enter_context(tc.tile_pool(name="nat", bufs=3))
        sqa_pool = ctx.enter_context(tc.tile_pool(name="sqa", bufs=2))
        sqb_pool = ctx.enter_context(tc.tile_pool(name="sqb", bufs=2))
        bt_pool = ctx.enter_context(tc.tile_pool(name="bt", bufs=2 * n_jtiles))
        rb_pool = ctx.enter_context(
            tc.tile_pool(name="rb", bufs=2 * max(n_jtiles // 2, 1))
        )
        at_pool = ctx.enter_context(tc.tile_pool(name="at", bufs=6))
        ra_pool = ctx.enter_context(tc.tile_pool(name="ra", bufs=8))
        tmp_pool = ctx.enter_context(tc.tile_pool(name="tmp", bufs=4))
        out_pool = ctx.enter_context(tc.tile_pool(name="out", bufs=8))
        # PSUM: 8 banks of [P, 512] fp32. 3x2 (pairs, shared by matmul
        # accumulation and the rB build) + 2x1 (transposes) = 8
        psum_pair = ctx.enter_context(tc.tile_pool(name="psum_pair", bufs=3, space="PSUM"))
        psum_tr = ctx.enter_context(tc.tile_pool(name="psum_tr", bufs=2, space="PSUM"))

        ident = const_pool.tile([P, P], F32)
        make_identity(nc, ident)
        # all-ones bf16 [P, P] (value 1.0): partition-reduction over the
        # bf16 squares of B^T produces broadcast rB
        ones_t = const_pool.tile([P, P], BF16)
        nc.scalar.activation(ones_t[:], ident[:], AF.Identity, bias=1.0, scale=0.0)
        # 1/128-valued bf16 [P, P]: partition-reduction over the already
        # broadcast rB reproduces rB (for the actp fold)
        ones_b = const_pool.tile([P, P], BF16)
        nc.scalar.mul(ones_b[:], ones_t[:], 1.0 / P)

        bt_chunks = {}  # (b, jt) -> [P, 2, NT] fp8 B^T chunk
        rb_pairs = {}  # (b, jp) -> [P, 2, NT] bf16 broadcast rB for jt 2jp, 2jp+1

        GW = LOADG * P  # j-width covered by one B group (== NT)
        n_bgroups = n_itiles // LOADG
        n_agroups = n_itiles // LOADG

        def emit_b_group(b, g):
            """Load + transpose one 512-wide B panel slice into an fp8
            chunk (DVE casts); square the (exact fp32) transposed tiles on
            ScalarE and reduce over partitions with an all-ones matmul to
            get broadcast rB."""
            bn = nat_pool.tile([P, LOADG, d], F32, tag="bn")
            nc.gpsimd.dma_start(
                bn[:],
                b_ext[b, ts(g, GW), :].rearrange("(t p) d -> p t d", p=P),
            )
            chunk = bt_pool.tile([P, n_ktiles, NT], FP8, tag="bt", name="bt_chunk")
            sqc = sqb_pool.tile([P, n_ktiles, NT], BF16, tag="sqb", name="sq_chunk")
            for tt in range(t_per_j):
                ps = psum_tr.tile([P, 2, P], F32, tag="ps_tr")
                for k in range(n_ktiles):
                    nc.tensor.transpose(ps[:, k, :], bn[:, tt, ts(k, P)], ident)
                # both k-chunks in one op each: DVE casts, ScalarE squares
                nc.vector.tensor_copy(chunk[:, :, ts(tt, P)], ps[:])
                nc.scalar.activation(sqc[:, :, ts(tt, P)], ps[:], AF.Square)
            jp, half = divmod(g, 2)
            if half == 0:
                rb_pairs[(b, jp)] = rb_pool.tile(
                    [P, 2, NT], BF16, tag="rb", name="rb_pair"
                )
            rb_ps = psum_pair.tile([P, 2 * NT], F32, tag="mm_pair", name="rb_ps")
            for k in range(n_ktiles):
                nc.tensor.matmul(
                    rb_ps[:, :NT],
                    lhsT=ones_t[:],
                    rhs=sqc[:, k, :],
                    start=(k == 0),
                    stop=(k == n_ktiles - 1),
                )
            nc.scalar.copy(rb_pairs[(b, jp)][:, half, :], rb_ps[:, :NT])
            bt_chunks[(b, g)] = chunk

        def load_a_group(b, g):
            t = nat_pool.tile([P, LOADG, d], F32, tag="an", name="an_group")
            nc.gpsimd.dma_start(
                t[:],
                a_ext[b, ts(g, LOADG * P), :].rearrange("(t p) d -> p t d", p=P),
            )
            return t

        def emit_a_row_pre(an):
            """rA square + A^T transpose and -2x fp8 cast for one row."""
            r_a = ra_pool.tile([P, 1], F32, tag="rA", name="r_a")
            sqa = sqa_pool.tile([P, d], BF16, tag="sqa")
            nc.scalar.activation(sqa[:], an, AF.Square, accum_out=r_a[:])
            at_tile = at_pool.tile([P, n_ktiles, P], FP8, tag="at", name="at_tile")
            ps = psum_tr.tile([P, 2, P], F32, tag="ps_tr")
            for k in range(n_ktiles):
                nc.tensor.transpose(ps[:, k, :], an[:, ts(k, P)], ident)
            # fold the -2 of "-2*a.b" into the fp8 cast of A^T (one op)
            nc.scalar.mul(at_tile[:, :, :], ps[:], -2.0)
            return r_a, at_tile

        def mm_cross(out_ps, b, jt, at_tile, start=True, stop=True, skip=False):
            """One DoubleRow fp8 matmul: full K=256 cross term for a j-tile."""
            nc.tensor.matmul(
                out_ps,
                lhsT=at_tile[:, :, :],
                rhs=bt_chunks[(b, jt)][:, :, :],
                start=start,
                stop=stop,
                perf_mode=mybir.MatmulPerfMode.DoubleRow,
                skip_group_check=skip,
            )

        def emit_item(b, item, r_a, at_tile, out_row):
            kind, j0 = item
            mm_ps = psum_pair.tile([P, 2 * NT], F32, tag="mm_pair", name="mm_pair")
            rbp = rb_pairs[(b, j0 // 2)]
            if kind == "dve1":  # tiny configs: single j-tile via DVE
                mm_cross(mm_ps[:, :NT], b, j0, at_tile)
                nc.vector.scalar_tensor_tensor(
                    out=out_row[:, ts(j0, NT)],
                    in0=mm_ps[:, :NT],
                    scalar=r_a[:],
                    in1=rbp[:, j0 % 2, :],
                    op0=ALU.add,
                    op1=ALU.add,
                )
                return
            if kind == "actp":
                # fold rB into PSUM: ones(1/128).T @ broadcast-rB per half
                for jj in range(2):
                    mm_cross(
                        mm_ps[:, ts(jj, NT)], b, j0 + jj, at_tile,
                        start=True, stop=False, skip=True,
                    )
                    nc.tensor.matmul(
                        mm_ps[:, ts(jj, NT)],
                        lhsT=ones_b[:],
                        rhs=rbp[:, jj, :],
                        start=False,
                        stop=True,
                        skip_group_check=True,
                    )
                nc.scalar.activation(
                    out_row[:, j0 * NT : (j0 + 2) * NT],
                    mm_ps[:],
                    AF.Identity,
                    bias=r_a[:],
                    scale=1.0,
                )
                return
            for jj in range(2):
                mm_cross(mm_ps[:, ts(jj, NT)], b, j0 + jj, at_tile)
            if kind == "dvep":
                nc.vector.scalar_tensor_tensor(
                    out=out_row[:, j0 * NT : (j0 + 2) * NT],
                    in0=mm_ps[:],
                    scalar=r_a[:],
                    in1=rbp[:, :, :],
                    op0=ALU.add,
                    op1=ALU.add,
                )
            else:  # "gpp": ScalarE evacuates psum with +rA; GpSimd adds rB
                tmp = tmp_pool.tile([P, 2 * NT], BF16, tag="tmp", name="act_tmp")
                nc.scalar.activation(
                    tmp[:], mm_ps[:], AF.Identity, bias=r_a[:], scale=1.0
                )
                nc.gpsimd.tensor_add(
                    out_row[:, j0 * NT : (j0 + 2) * NT],
                    tmp[:],
                    rbp[:, :, :].rearrange("p two n -> p (two n)"),
                )

        an_groups = {0: load_a_group(0, 0)}

        # --- batch-0 startup: first LOADG rows emitted j-outer, interleaved
        # with the B preprocess, so output DMAs start as soon as chunks land.
        pre_rows = min(LOADG, n_itiles)
        pre = [emit_a_row_pre(an_groups[0][:, r]) for r in range(pre_rows)]
        if n_agroups > 1 or b_per_core > 1:
            gnext = 1 % n_agroups
            an_groups[gnext] = load_a_group(0 if n_agroups > 1 else 1, gnext)
        pre_outs = [
            out_pool.tile([P, n], BF16, tag="out_row", name="out_row")
            for _ in range(pre_rows)
        ]
        # warmup rows r use plans[r % len(plans)]; emit each item as soon as
        # its last B chunk (group j0+1, or j0 for single) is processed
        for g in range(n_bgroups):
            emit_b_group(0, g)
            for r in range(pre_rows):
                for item in plans[r % len(plans)]:
                    last_g = item[1] + (0 if item[0] == "dve1" else 1)
                    if last_g == g:
                        emit_item(0, item, pre[r][0], pre[r][1], pre_outs[r])
        for r in range(pre_rows):
            nc.sync.dma_start(d_ext[0, ts(r, P), :], pre_outs[r][:])

        # --- main loop
        b_emitted = {0: n_bgroups}  # batch -> number of B groups emitted
        for b in range(b_per_core):
            for g in range(b_emitted.get(b, 0), n_bgroups):
                emit_b_group(b, g)  # catch-up (only for tiny configs)
                b_emitted[b] = g + 1
            for it in range(pre_rows if b == 0 else 0, n_itiles):
                # spread next batch's B preprocess across early iterations
                if b + 1 < b_per_core:
                    it0 = it - (pre_rows if b == 0 else 0)
                    if it0 < n_bgroups:
                        emit_b_group(b + 1, it0)
                        b_emitted[b + 1] = it0 + 1

                g, ti = divmod(it, LOADG)
                if ti == 0:
                    # prefetch the next A group one group ahead
                    if g + 1 < n_agroups:
                        an_groups[g + 1] = load_a_group(b, g + 1)
                    elif b + 1 < b_per_core:
                        an_groups[0] = load_a_group(b + 1, 0)
                an = an_groups[g][:, ti]
                r_a, at_tile = emit_a_row_pre(an)
                out_row = out_pool.tile([P, n], BF16, tag="out_row")
                for item in plans[it % len(plans)]:
                    emit_item(b, item, r_a, at_tile, out_row)
                nc.sync.dma_start(d_ext[b, ts(it, P), :], out_row[:])

    nc.compile()
    return nc


_NC_CACHE = {}


def _get_nc(b_per_core, n, d):
    key = (b_per_core, n, d)
    if key not in _NC_CACHE:
        _NC_CACHE[key] = build_nc(b_per_core, n, d)
    return _NC_CACHE[key]


def run(A, B, trace=False, trace_kwargs=None):
    """Run on hardware across 8 cores; returns (D_full, BassKernelResults)."""
    from concourse.bass_utils import run_bass_kernel_spmd

    A = np.ascontiguousarray(np.asarray(A, dtype=np.float32))
    B = np.ascontiguousarray(np.asarray(B, dtype=np.float32))
    full_b = A.shape[0]
    assert full_b % N_CORES == 0
    bpc = full_b // N_CORES
    nc = _get_nc(bpc, A.shape[1], A.shape[2])

    in_maps = [
        {
            "A": A[c * bpc : (c + 1) * bpc],
            "B": B[c * bpc : (c + 1) * bpc],
        }
        for c in range(N_CORES)
    ]
    res = run_bass_kernel_spmd(
        nc,
        in_maps,
        list(range(N_CORES)),
        trace=trace,
        **(trace_kwargs or {}),
    )
    out = np.concatenate(
        [np.asarray(r["D"]).astype(np.float32) for r in res.results], axis=0
    )
    return out, res


def kernel(A, B):
    out, _ = run(A, B, trace=False)
    return out
